# revision 1
# baseline (speedup 1.0000x reference)
"""CrossNetwork (DCN) forward on 8 TRN2 NeuronCores.

Reference computation (per cross layer i, x0 = input):
    s_i = xl . w_i            (per-row scalar)
    xl  = x0 * s_i + b_i + xl

Algebraic collapse: xl_i = alpha_i * x0 + c_i with per-row scalar alpha_i
and a row-constant vector c_i = sum_{j<i} b_j. Hence:
    u_i       = x0 . w_i                      (3 dots per row, all vs x0)
    alpha_0   = 1,  alpha_{i+1} = alpha_i * (1 + u_i) + (c_i . w_i)
    out       = alpha_3 * x0 + c_3
One read of x, one write of out -> memory roofline.

Sharding: pure data parallel over the batch dim, weights replicated.

Two NEFF variants are built lazily: a general one, and a b == 0
specialization (c_i = 0, k_i = 0) that needs no bias constants and spreads
compute across DVE / Pool / ACT and DMA across the SP/ACT HWDGE queues and
Pool's SWDGE so no engine exceeds the memory roofline. The right variant is
picked at run time by inspecting b (the reference always passes b = 0).
"""

import contextlib

import numpy as np

import concourse.bacc as bacc
import concourse.mybir as mybir
import concourse.tile as tile
from concourse.bass_utils import run_bass_kernel_spmd

N_CORES = 8
B, D, CROSS = 16384, 2048, 3
P = 128
F32 = mybir.dt.float32


def build_body_zero_b(tc, x_ap, w_ap, b_ap, out_ap, rows, reps=1):
    """b == 0 specialization: out = alpha3 * x, alpha3 = (1+u0)(1+u1)(1+u2).

    Work is spread so no engine exceeds the DMA roofline: dots 0/1 on DVE
    (scalar_tensor_tensor + accum), dot 2 as Pool multiply + ACT
    accumulate-copy, recurrence + final scale on ACT (2 of 16 finals on
    DVE), and the 32 MiB of tile DMA split across the SP + ACT HWDGE
    queues and Pool's SWDGE.

    reps > 1 repeats the main loop in-NEFF (benchmarking only).
    """
    nc = tc.nc
    nt = rows // P
    Al = mybir.AluOpType
    Act = mybir.ActivationFunctionType

    with contextlib.ExitStack() as ctx:
        const = ctx.enter_context(tc.tile_pool(name="const", bufs=1))
        xpool = ctx.enter_context(tc.tile_pool(name="x", bufs=6))
        ypool = ctx.enter_context(tc.tile_pool(name="y", bufs=6))
        spool = ctx.enter_context(tc.tile_pool(name="scr", bufs=2))
        sppool = ctx.enter_context(tc.tile_pool(name="scrp", bufs=3))
        sapool = ctx.enter_context(tc.tile_pool(name="scra", bufs=2))
        upool = ctx.enter_context(tc.tile_pool(name="u", bufs=24))

        # Replicate w_i across partitions with direct stride-0 DMA reads of
        # the DRAM row, one per issue engine, so all three broadcasts land in
        # ~one transfer time and no compute engine serializes behind them.
        # (w2 on ACT: Pool's dot-2 multiply consumes it first.)
        wbc = []
        for i, eng in [(2, nc.scalar), (0, nc.sync), (1, nc.gpsimd)]:
            wt = const.tile([P, D], F32, tag=f"w{i}")
            eng.dma_start(out=wt[:], in_=w_ap[i : i + 1, :].to_broadcast([P, D]))
            wbc.append((i, wt))
        wbc = [t for _, t in sorted(wbc)]

        # DMA issue assignment per tile: spread transfer time across the two
        # HWDGE engines (SP, ACT) and Pool's SWDGE so no single issue stream
        # carries the whole 32MiB.
        load_eng = {2: nc.gpsimd, 6: nc.gpsimd, 10: nc.gpsimd, 14: nc.gpsimd}
        store_eng = {}
        for i in (1, 5, 9):
            store_eng[i] = nc.scalar
        for i in (0, 4, 8, 12, 14):
            store_eng[i] = nc.gpsimd

        for t in range(nt * reps):
            t = t % nt
            xt = xpool.tile([P, D], F32, tag="x")
            load_eng.get(t % 16, nc.sync).dma_start(
                out=xt[:], in_=x_ap[t * P : (t + 1) * P, :]
            )

            # u0, u1 on DVE
            u0 = upool.tile([P, 1], F32, tag="u0")
            scr0 = spool.tile([P, D], F32, tag="scr")
            nc.vector.scalar_tensor_tensor(
                out=scr0[:], in0=xt[:], scalar=0.0, in1=wbc[0][:],
                op0=Al.bypass, op1=Al.mult, accum_out=u0[:],
            )
            u1 = upool.tile([P, 1], F32, tag="u1")
            scr1 = spool.tile([P, D], F32, tag="scr")
            nc.vector.scalar_tensor_tensor(
                out=scr1[:], in0=xt[:], scalar=0.0, in1=wbc[1][:],
                op0=Al.bypass, op1=Al.mult, accum_out=u1[:],
            )
            # u2 = sum(x * w2): multiply on Pool, accumulate on ACT
            scrp = sppool.tile([P, D], F32, tag="scrp")
            nc.gpsimd.tensor_tensor(out=scrp[:], in0=xt[:], in1=wbc[2][:],
                                    op=Al.mult)
            u2 = upool.tile([P, 1], F32, tag="u2")
            scra = sapool.tile([P, D], F32, tag="scra")
            nc.scalar.activation(scra[:], scrp[:], Act.Copy, accum_out=u2[:])

            # alpha3 = (1+u0)(1+u1)(1+u2) on ACT
            t1 = upool.tile([P, 1], F32, tag="t1")
            nc.scalar.add(t1[:], u0[:], 1.0)
            t2 = upool.tile([P, 1], F32, tag="t2")
            nc.scalar.add(t2[:], u1[:], 1.0)
            a2 = upool.tile([P, 1], F32, tag="a2")
            nc.scalar.activation(a2[:], t2[:], Act.Identity, bias=0.0, scale=t1[:])
            t3 = upool.tile([P, 1], F32, tag="t3")
            nc.scalar.add(t3[:], u2[:], 1.0)
            a3 = upool.tile([P, 1], F32, tag="a3")
            nc.scalar.activation(a3[:], t3[:], Act.Identity, bias=0.0, scale=a2[:])

            # out = alpha3 * x0: mostly ACT, two tiles per 16 on DVE (2x
            # fp32 tensor_scalar) to trim the ACT span.
            yt = ypool.tile([P, D], F32, tag="y")
            if t % 4 == 3:
                nc.vector.tensor_scalar_mul(yt[:], xt[:], a3[:])
            else:
                nc.scalar.activation(yt[:], xt[:], Act.Copy, scale=a3[:])
            store_eng.get(t % 16, nc.sync).dma_start(
                out=out_ap[t * P : (t + 1) * P, :], in_=yt[:]
            )


def build_body_general(tc, x_ap, w_ap, b_ap, out_ap, rows):
    """General-b path: full constants, final = ACT scale + Pool bias-add."""
    nc = tc.nc
    nt = rows // P
    Al = mybir.AluOpType
    Act = mybir.ActivationFunctionType

    with contextlib.ExitStack() as ctx:
        const = ctx.enter_context(tc.tile_pool(name="const", bufs=1))
        xpool = ctx.enter_context(tc.tile_pool(name="x", bufs=4))
        ypool = ctx.enter_context(tc.tile_pool(name="y", bufs=4))
        spool = ctx.enter_context(tc.tile_pool(name="scr", bufs=3))
        upool = ctx.enter_context(tc.tile_pool(name="u", bufs=16))

        # Load each tiny w_i / b_i row to partition 0, then replicate across
        # all 128 partitions on-chip (gpsimd partition_broadcast). The custom
        # op requires its input AP to start at partition 0, hence one [1, D]
        # tile per row. All row tiles are transient (pre pool).
        with tc.tile_pool(name="pre", bufs=1) as pre:
            wrow = []
            brow = []
            for i in range(CROSS):
                wr = pre.tile([1, D], F32, tag=f"wr{i}")
                nc.sync.dma_start(out=wr[:], in_=w_ap[i : i + 1, :])
                wrow.append(wr)
                br = pre.tile([1, D], F32, tag=f"br{i}")
                nc.sync.dma_start(out=br[:], in_=b_ap[i : i + 1, :])
                brow.append(br)

            wbc = []
            for i in range(CROSS):
                wt = const.tile([P, D], F32, tag=f"w{i}")
                nc.gpsimd.partition_broadcast(wt[:], wrow[i][:])
                wbc.append(wt)

            # row constants on [1, D]: c2 = b0 + b1, c3 = c2 + b2
            c2row = pre.tile([1, D], F32, tag="c2r")
            nc.vector.tensor_add(c2row[:], brow[0][:], brow[1][:])
            c3row = pre.tile([1, D], F32, tag="c3r")
            nc.vector.tensor_add(c3row[:], c2row[:], brow[2][:])
            c3bc = const.tile([P, D], F32, tag="c3")
            nc.gpsimd.partition_broadcast(c3bc[:], c3row[:])

            # k1 = b0 . w1, k2 = c2 . w2 (scalars), then replicate to [P, 1]
            k1row = pre.tile([1, 1], F32, tag="k1r")
            scr_k1 = pre.tile([1, D], F32, tag="scrr")
            nc.vector.scalar_tensor_tensor(
                out=scr_k1[:], in0=brow[0][:], scalar=0.0, in1=wrow[1][:],
                op0=Al.bypass, op1=Al.mult, accum_out=k1row[:],
            )
            k2row = pre.tile([1, 1], F32, tag="k2r")
            scr_k2 = pre.tile([1, D], F32, tag="scrr2")
            nc.vector.scalar_tensor_tensor(
                out=scr_k2[:], in0=c2row[:], scalar=0.0, in1=wrow[2][:],
                op0=Al.bypass, op1=Al.mult, accum_out=k2row[:],
            )
            k1bc = const.tile([P, 1], F32, tag="k1")
            nc.gpsimd.partition_broadcast(k1bc[:], k1row[:])
            k2bc = const.tile([P, 1], F32, tag="k2")
            nc.gpsimd.partition_broadcast(k2bc[:], k2row[:])

        for t in range(nt):
            xt = xpool.tile([P, D], F32, tag="x")
            nc.sync.dma_start(out=xt[:], in_=x_ap[t * P : (t + 1) * P, :])

            us = []
            for i in range(CROSS):
                u = upool.tile([P, 1], F32, tag=f"u{i}")
                scr = spool.tile([P, D], F32, tag="scr")
                nc.vector.scalar_tensor_tensor(
                    out=scr[:], in0=xt[:], scalar=0.0, in1=wbc[i][:],
                    op0=Al.bypass, op1=Al.mult, accum_out=u[:],
                )
                us.append(u)

            # alpha recurrence on ACT: a3 = ((1+u0)(1+u1) + k1)(1+u2) + k2
            t1 = upool.tile([P, 1], F32, tag="t1")
            nc.scalar.add(t1[:], us[0][:], 1.0)
            t2 = upool.tile([P, 1], F32, tag="t2")
            nc.scalar.add(t2[:], us[1][:], 1.0)
            a2 = upool.tile([P, 1], F32, tag="a2")
            nc.scalar.activation(a2[:], t2[:], Act.Identity, bias=k1bc[:], scale=t1[:])
            t3 = upool.tile([P, 1], F32, tag="t3")
            nc.scalar.add(t3[:], us[2][:], 1.0)
            a3 = upool.tile([P, 1], F32, tag="a3")
            nc.scalar.activation(a3[:], t3[:], Act.Identity, bias=k2bc[:], scale=a2[:])

            # out = alpha3 * x0 + c3: scale on ACT, bias-add in place on Pool
            yt = ypool.tile([P, D], F32, tag="y")
            nc.scalar.activation(yt[:], xt[:], Act.Copy, scale=a3[:])
            nc.gpsimd.tensor_tensor(out=yt[:], in0=yt[:], in1=c3bc[:], op=Al.add)
            nc.sync.dma_start(out=out_ap[t * P : (t + 1) * P, :], in_=yt[:])


_CACHE = {}


def get_nc(rows, zero_b=False, reps=1):
    key = (rows, zero_b, reps)
    if key not in _CACHE:
        nc = bacc.Bacc(
            "TRN2",
            target_bir_lowering=False,
            debug=False,
            enable_asserts=False,
            num_devices=N_CORES,
        )
        x = nc.dram_tensor("x", [rows, D], F32, kind="ExternalInput").ap()
        w = nc.dram_tensor("W", [CROSS, D], F32, kind="ExternalInput").ap()
        b = nc.dram_tensor("b", [CROSS, D], F32, kind="ExternalInput").ap()
        out = nc.dram_tensor("out", [rows, D], F32, kind="ExternalOutput").ap()
        with tile.TileContext(nc) as tc:
            if zero_b:
                build_body_zero_b(tc, x, w, b, out, rows, reps=reps)
            else:
                build_body_general(tc, x, w, b, out, rows)
        nc.compile()
        _CACHE[key] = nc
    return _CACHE[key]


def run(x, W, b, trace=False, force_general=False):
    x = np.ascontiguousarray(np.asarray(x, dtype=np.float32))
    W = np.ascontiguousarray(np.asarray(W, dtype=np.float32))
    b = np.ascontiguousarray(np.asarray(b, dtype=np.float32))
    rows = x.shape[0] // N_CORES
    zero_b = (not force_general) and not b.any()
    nc = get_nc(rows, zero_b)
    in_maps = [
        {"x": x[i * rows : (i + 1) * rows], "W": W, "b": b} for i in range(N_CORES)
    ]
    try:
        res = run_bass_kernel_spmd(
            nc, in_maps, core_ids=list(range(N_CORES)), trace=trace
        )
    except ModuleNotFoundError:
        # BASS_TRACE in the environment routes through an NTFF profile hook
        # that is absent in some containers; fall back to an untraced run.
        import os

        os.environ["BASS_NEVER_TRACE"] = "1"
        res = run_bass_kernel_spmd(
            nc, in_maps, core_ids=list(range(N_CORES)), trace=False
        )
    out = np.concatenate([r["out"] for r in res.results], axis=0)
    return out, res


def kernel(x, W, b):
    out, _ = run(x, W, b)
    return out

